# revision 90
# baseline (speedup 1.0000x reference)
"""GCN (2-layer GCNConv + log_softmax) on 8 Trainium2 NeuronCores.

Strategy:
  - Nodes sharded by id range across 8 cores (12500/core); edges sharded by
    dst.  Host preprocessing is index-only: sort edges by dst, deal each
    core's nodes (sorted by degree) onto 128 partitions x 98 rows, pad each
    row to a cross-core common degree D_common[r], and emit ONE gather-slot
    index/selector stream shared by both layers.  Self-loop edges are
    excluded from the streams; their contribution is added densely.
  - x is fed per-core in DEALT order and TRANSPOSED ([128 feats, NPAD]) so
    phase 1 uses it directly as matmul lhsT (no PE transposes).
  - Gather tables are bf16 rows padded to 64B (16 or 8 feats + pad) inside
    f32-typed containers (the cost of a dma_gather scales with elem COUNT,
    not bytes, so 64 f32-typed elems beat 128 bf16 elems for the same
    256B).  One 256B gather element = 4 consecutive dealt node rows; a
    host-precomputed one-hot mask selects the right row.  The mask ships
    duplicated in adjacent bf16 pairs so every operand of the mask-multiply
    keeps a packed stride-1 innermost dim -- that qualifies it for the DVE
    2x 16-bit perf mode (the feature broadcast rides a stride-0 MIDDLE dim).
  - Per-edge compute: 2x mask-multiply, then a bf16 add-tree folds the 4
    candidate rows (also 2x), then one strided tensor_reduce per run of
    equal-D rows (16/8 elems per edge instead of 64).  Each call's
    multiply/tree is column-split between DVE and GpSimd (POOL_FRAC_L1/L2)
    to balance the engines; PSUM->SBUF copies, scaling, and relu run on the
    Activation engine.  Calls run thin-rows-first with a small leading call
    so the gather pipeline primes quickly at both layer starts.
  - Device program (single SPMD NEFF, Tile-scheduled):
      g1 = (x @ W1) * dinv  (PE, batched PSUM tiles; kept in SBUF)
      AllGather g1 -> global dealt-order table
      layer-1 aggregation (gather + mask + tree + reduce), interleaved with
        per-row-range glue h1 = relu((agg + g1)*dinv + b1) and phase 4
        (g2 = (h1 @ W2) * dinv) so the second matmul hides under the
        layer-1 gathers
      AllGather g2 ; layer-2 aggregation ; out = log_softmax((agg2+g2)*dinv+b2)
  - dinv[src] is folded into the tables, dinv[dst] applied after
    aggregation, so no per-edge norm array exists; the self-loop message is
    exactly the table row g[n], added before the dinv[dst] scale.
"""

import numpy as np

N = 100000
FIN = 128
HID = 16
NCLS = 8
NCORES = 8
NLOC = N // NCORES          # 12500
P = 128
R = (NLOC + P - 1) // P     # 98
NPAD = R * P                # 12544
CTILE = 96                  # max gather chunks (of 128 slots) per dma_gather call
POOL_FRAC_L1 = 0.18         # mask-mult fraction offloaded to GpSimd, layer 1
POOL_FRAC_L2 = 0.0         # layer 2 (half the mask work -> less offload)

_cache = {}


def _build_program(D_common, calls):
    import concourse.bacc as bacc
    import concourse.mybir as mybir
    import concourse.tile as tile
    from concourse.library_config import mlp as mlp_lib

    c0 = np.concatenate([[0], np.cumsum(D_common)]).astype(np.int64)
    CT = int(c0[-1])
    f32 = mybir.dt.float32
    i16 = mybir.dt.int16
    add = mybir.AluOpType.add
    mult = mybir.AluOpType.mult

    nc = bacc.Bacc("TRN2", target_bir_lowering=False, debug=False, num_devices=NCORES)

    def T(name, shape, dt, kind):
        return nc.dram_tensor(name, shape, dt, kind=kind).ap()

    bf16_ = mybir.dt.bfloat16
    xt_in = T("xt", [FIN, NPAD], bf16_, "ExternalInput")
    deg_in = T("deg", [P, R], f32, "ExternalInput")
    idx_in = T("idx", [P, 8 * CT], i16, "ExternalInput")
    ms_in = T("ms", [P, 8 * CT], bf16_, "ExternalInput")
    w1_in = T("w1", [FIN, HID], bf16_, "ExternalInput")
    b1_in = T("b1", [P, HID], f32, "ExternalInput")
    w2_in = T("w2", [HID, NCLS], f32, "ExternalInput")
    b2_in = T("b2", [P, NCLS], f32, "ExternalInput")
    out_t = T("out", [P, R, NCLS], f32, "ExternalOutput")


    with tile.TileContext(nc) as tc:
        with (
            tc.tile_pool(name="persist", bufs=1) as pp,
            tc.tile_pool(name="xload", bufs=3) as xp,
            tc.tile_pool(name="psum", bufs=2, space="PSUM") as psp,
            tc.tile_pool(name="small", bufs=3) as sp,
            tc.tile_pool(name="gidx", bufs=4) as gip,
            tc.tile_pool(name="gbuf", bufs=3) as gbp,
            tc.tile_pool(name="gprod", bufs=2) as gpp,
            tc.tile_pool(name="dram", bufs=1, space="DRAM") as dp,
        ):
            nc.gpsimd.load_library(mlp_lib)

            w1_t = pp.tile([FIN, HID], mybir.dt.bfloat16)
            nc.sync.dma_start(w1_t[:], w1_in)
            b1_t = pp.tile([P, HID], f32)
            nc.sync.dma_start(b1_t[:], b1_in)
            w2_t = pp.tile([HID, NCLS], f32)
            nc.sync.dma_start(w2_t[:], w2_in)
            b2_t = pp.tile([P, NCLS], f32)
            nc.sync.dma_start(b2_t[:], b2_in)
            deg = pp.tile([P, R], f32)
            nc.sync.dma_start(deg[:], deg_in)
            sq = pp.tile([P, R], f32)
            dinv = pp.tile([P, R], f32)
            nc.scalar.sqrt(sq[:], deg[:])
            nc.vector.reciprocal(dinv[:], sq[:])

            bf16 = mybir.dt.bfloat16
            # tables are f32-typed byte containers; rows hold 32 bf16 values
            # (16/8 features + pad) so one 256B gather elem = 4 node rows
            RW = 16   # f32 elems per table row (= 32 bf16 = 64B)
            g1_loc = dp.tile([NPAD, RW], f32)
            g1_full = dp.tile([NCORES * NPAD // 4, 4 * RW], f32)   # [25088, 64]
            g2_loc = dp.tile([NPAD, RW], f32)
            g2_full = dp.tile([NCORES * NPAD // 4, 4 * RW], f32)   # [25088, 64]

            # ---- phase 1: g1 = (x @ W1) * dinv, dealt order; keep in SBUF ----
            g1u = pp.tile([P, R, HID], f32)
            g1_sb = pp.tile([P, R, HID], f32)
            copyf = mybir.ActivationFunctionType.Copy
            XCH = 14                      # dealt-node chunks per xt DMA
            PCH = 7                       # chunks batched per PSUM tile
            HR = R // 2                   # rows per table half
            g1bf = pp.tile([P, R, HID], bf16)
            g1_loc_v = g1_loc.bitcast(bf16)[:, :HID].rearrange(
                "(r p) f -> p r f", p=P)

            def ag_full(loc, full):
                nc.gpsimd.collective_compute(
                    "AllGather", mybir.AluOpType.bypass,
                    replica_groups=[list(range(NCORES))],
                    ins=[loc.opt()], outs=[full.opt()],
                )

            for h_ in range(2):
                hr0, hr1 = h_ * HR, (h_ + 1) * HR
                for cb_ in range(hr0, hr1, XCH):
                    nch = min(XCH, hr1 - cb_)
                    xt = xp.tile([FIN, XCH * P], bf16, tag="xt")
                    nc.sync.dma_start(
                        xt[:, :nch * P], xt_in[:, cb_ * P:(cb_ + nch) * P])
                    for pb_ in range(0, nch, PCH):
                        npc = min(PCH, nch - pb_)
                        ps_h = psp.tile([P, PCH * HID], f32, space="PSUM")
                        for ci_ in range(npc):
                            nc.tensor.matmul(
                                ps_h[:, ci_ * HID:(ci_ + 1) * HID],
                                lhsT=xt[:, (pb_ + ci_) * P:(pb_ + ci_ + 1) * P],
                                rhs=w1_t[:], start=True, stop=True)
                        nc.scalar.activation(
                            g1u[:, cb_ + pb_:cb_ + pb_ + npc, :],
                            ps_h[:, :npc * HID].rearrange(
                                "p (c f) -> p c f", c=npc),
                            copyf)
                nc.vector.tensor_tensor(
                    out=g1_sb[:, hr0:hr1, :], in0=g1u[:, hr0:hr1, :],
                    in1=dinv[:, hr0:hr1].unsqueeze(2).to_broadcast(
                        [P, HR, HID]), op=mult)
                nc.vector.tensor_copy(g1bf[:, hr0:hr1, :], g1_sb[:, hr0:hr1, :])
                nc.sync.dma_start(g1_loc_v[:, hr0:hr1, :], g1bf[:, hr0:hr1, :])
            ag_full(g1_loc, g1_full)

            NJ = 4

            def issue_gather(table, ci):
                # idx/ms loads + gather for call ci; issued one call ahead of
                # its compute so pool-offloaded multiplies never sit between
                # consecutive descriptor generations in Pool's in-order queue
                r0, r1, cc = calls[ci]
                cb = int(c0[r0])
                idxt = gip.tile([P, 8 * CTILE], i16, tag="gidx")
                nc.sync.dma_start(idxt[:, :8 * cc], idx_in[:, 8 * cb:8 * (cb + cc)])
                buf = gbp.tile([P, CTILE, NJ * RW], f32, tag="gbuf")
                nc.gpsimd.dma_gather(
                    buf[:, :cc, :],
                    table[:],
                    idxt[:, :8 * cc], cc * 128, cc * 128, NJ * RW,
                    single_packet=False,
                )
                mst = sp.tile([P, 8 * CTILE], bf16, tag="mst")
                nc.sync.dma_start(mst[:, :8 * cc], ms_in[:, 8 * cb:8 * (cb + cc)])
                return buf, mst

            def aggregate(table, nf, out_tile, frac, post_cb=None):
                pending = issue_gather(table, 0)
                for ci, (r0, r1, cc) in enumerate(calls):
                    cb = int(c0[r0])
                    buf, mst = pending
                    if ci + 1 < len(calls):
                        pending = issue_gather(table, ci + 1)
                    buf_bf = buf[:].bitcast(bf16)   # [P, CTILE, NJ*2*RW] bf16
                    # mask ships duplicated in adjacent pairs so the multiply
                    # keeps a packed stride-1 innermost dim on every operand
                    # (DVE 2x 16-bit perf mode needs it; the broadcast over
                    # feature PAIRS uses a stride-0 middle dim instead)
                    mt = mst.rearrange("p (c j d) -> p c j d", j=NJ, d=2)
                    prod = gpp.tile([P, CTILE, NJ * 16], bf16, tag="gprod")
                    # split each call's mult+tree by columns between DVE and
                    # GpSimd: both engines work the call concurrently and the
                    # ratio is continuous (no whole-call lumps blocking the
                    # next descriptor generation on Pool)
                    s_ = cc - int(round(cc * frac))
                    spans = [(nc.vector, 0, s_), (nc.gpsimd, s_, cc)]
                    for eng, cl, ch_ in spans:
                        if ch_ <= cl:
                            continue
                        ncol = ch_ - cl
                        eng.tensor_tensor(
                            out=prod[:, cl:ch_, :NJ * nf].rearrange(
                                "p c (j g d) -> p c j g d", j=NJ, d=2),
                            in0=buf_bf[:, cl:ch_, :].rearrange(
                                "p c (j w) -> p c j w", j=NJ)[:, :, :, :nf]
                                .rearrange("p c j (g d) -> p c j g d", d=2),
                            in1=mt[:, cl:ch_, :, :].unsqueeze(3).to_broadcast(
                                [P, ncol, NJ, nf // 2, 2]),
                            op=mult,
                        )
                    # fold the j axis with a bf16 add-tree (2x DVE perf mode:
                    # all operands 2-byte, packed innermost) down to [P,cc,nf]
                    w = NJ
                    src_t, src_w = prod, NJ * nf
                    while w > 1:
                        w //= 2
                        dstt = gpp.tile([P, CTILE, w * nf], bf16,
                                        tag=f"tree{w}_{nf}")
                        for eng, cl, ch_ in spans:
                            if ch_ <= cl:
                                continue
                            sv = src_t[:, cl:ch_, :src_w].rearrange(
                                "p c (u j f) -> p c u j f", u=w, j=2)
                            eng.tensor_tensor(
                                out=dstt[:, cl:ch_, :].rearrange(
                                    "p c (u f) -> p c u f", u=w),
                                in0=sv[:, :, :, 0, :],
                                in1=sv[:, :, :, 1, :],
                                op=add,
                            )
                        src_t, src_w = dstt, w * nf
                    r = r0
                    while r < r1:
                        d = int(D_common[r])
                        r2 = r
                        while r2 < r1 and int(D_common[r2]) == d:
                            r2 += 1
                        nr = r2 - r
                        lc = int(c0[r]) - cb
                        sl = src_t[:, lc:lc + nr * d, :nf].rearrange(
                            "p (n c) f -> p n c f", n=nr)
                        nc.vector.tensor_reduce(
                            out=out_tile[:, r:r2, :],
                            in_=sl.transpose([0, 1, 3, 2]),
                            axis=mybir.AxisListType.X,
                            op=add,
                        )
                        r = r2
                    if post_cb is not None:
                        post_cb(r0, r1)

            # ---- phase 3: layer-1 aggregation (glue + phase 4 interleaved) ----
            from concourse.masks import make_identity
            ident = pp.tile([P, P], f32)
            make_identity(nc, ident[:])
            h1a = pp.tile([P, R, HID], f32)
            h1 = pp.tile([P, R, HID], f32)
            g2_sb = pp.tile([P, R, NCLS], f32)

            l1_state = {"half0": False}

            def layer1_post(r0, r1):
                nr = r1 - r0
                # self-loop msg is g1_sb (= (xW1)*dinv);
                # h1 = relu((h1a + g1_sb) * dinv + b1)
                hs = h1[:, r0:r1, :]
                nc.vector.tensor_tensor(
                    out=hs, in0=h1a[:, r0:r1, :], in1=g1_sb[:, r0:r1, :], op=add)
                nc.vector.tensor_tensor(
                    out=hs, in0=hs,
                    in1=dinv[:, r0:r1].unsqueeze(2).to_broadcast([P, nr, HID]),
                    op=mult)
                nc.vector.tensor_tensor(
                    out=hs, in0=hs,
                    in1=b1_t[:].unsqueeze(1).to_broadcast([P, nr, HID]), op=add)
                nc.scalar.activation(hs, hs, mybir.ActivationFunctionType.Relu)
                # phase 4 for these rows: g2 = (h1 @ W2) * dinv
                for ch in range(r0, r1):
                    ps_ht = psp.tile([HID, P], f32, space="PSUM")
                    nc.tensor.transpose(ps_ht[:], h1[:, ch, :], ident[:])
                    h1T = sp.tile([HID, P], f32, tag="h1T")
                    nc.scalar.activation(h1T[:], ps_ht[:], copyf)
                    ps_u = psp.tile([P, NCLS], f32, space="PSUM")
                    nc.tensor.matmul(
                        ps_u[:], lhsT=h1T[:], rhs=w2_t[:], start=True, stop=True)
                    nc.scalar.activation(
                        g2_sb[:, ch, :], ps_u[:], copyf, scale=dinv[:, ch:ch + 1])
                nc.scalar.activation(
                    g2bf[:, r0:r1, :], g2_sb[:, r0:r1, :], copyf)
                nc.sync.dma_start(
                    g2_loc.bitcast(bf16)[:, :NCLS].rearrange(
                        "(r p) f -> p r f", p=P)[:, r0:r1, :],
                    g2bf[:, r0:r1, :])


            g2bf = pp.tile([P, R, NCLS], bf16)
            aggregate(g1_full, HID, h1a, POOL_FRAC_L1, post_cb=layer1_post)
            ag_full(g2_loc, g2_full)

            # ---- phase 6: layer-2 aggregation; glue+softmax+output emitted in
            # two halves so the first half hides under the remaining gathers
            o2a = pp.tile([P, R, NCLS], f32)
            o2 = pp.tile([P, R, NCLS], f32)
            mx = pp.tile([P, R], f32)
            tm = pp.tile([P, R, NCLS], f32)
            ex = pp.tile([P, R, NCLS], f32)
            se = pp.tile([P, R], f32)
            lse = pp.tile([P, R], f32)
            res = pp.tile([P, R, NCLS], f32)

            def finish_rows(q0, q1):
                nq = q1 - q0
                os_ = o2[:, q0:q1, :]
                nc.vector.tensor_tensor(
                    out=os_, in0=o2a[:, q0:q1, :], in1=g2_sb[:, q0:q1, :], op=add)
                nc.vector.tensor_tensor(
                    out=os_, in0=os_,
                    in1=dinv[:, q0:q1].unsqueeze(2).to_broadcast([P, nq, NCLS]),
                    op=mult)
                nc.vector.tensor_tensor(
                    out=os_, in0=os_,
                    in1=b2_t[:].unsqueeze(1).to_broadcast([P, nq, NCLS]), op=add)
                nc.vector.tensor_reduce(
                    out=mx[:, q0:q1], in_=os_, axis=mybir.AxisListType.X,
                    op=mybir.AluOpType.max)
                nc.vector.tensor_tensor(
                    out=tm[:, q0:q1, :], in0=os_,
                    in1=mx[:, q0:q1].unsqueeze(2).to_broadcast([P, nq, NCLS]),
                    op=mybir.AluOpType.subtract)
                nc.scalar.activation(
                    ex[:, q0:q1, :], tm[:, q0:q1, :],
                    mybir.ActivationFunctionType.Exp)
                nc.vector.tensor_reduce(
                    out=se[:, q0:q1], in_=ex[:, q0:q1, :],
                    axis=mybir.AxisListType.X, op=add)
                nc.scalar.activation(
                    lse[:, q0:q1], se[:, q0:q1],
                    mybir.ActivationFunctionType.Ln)
                nc.vector.tensor_tensor(
                    out=res[:, q0:q1, :], in0=tm[:, q0:q1, :],
                    in1=lse[:, q0:q1].unsqueeze(2).to_broadcast([P, nq, NCLS]),
                    op=mybir.AluOpType.subtract)
                nc.sync.dma_start(out_t[:, q0:q1, :], res[:, q0:q1, :])

            l2_pending = [(2 * R // 3, R), (R // 3, 2 * R // 3)]

            def layer2_post(r0, r1):
                while l2_pending and r0 <= l2_pending[0][0]:
                    q0, q1 = l2_pending.pop(0)
                    finish_rows(q0, q1)

            aggregate(g2_full, NCLS, o2a, POOL_FRAC_L2, post_cb=layer2_post)
            finish_rows(0, R // 3)

    nc.compile()
    return nc


def _preprocess(x, edge_index):
    src = edge_index[0].astype(np.int64)
    dst = edge_index[1].astype(np.int64)
    # degrees include the self-loop (reference adds them before normalizing)
    deg = np.bincount(dst, minlength=N).astype(np.int64) + 1
    order = np.argsort(dst, kind="stable")
    ssrc = src[order]                         # srcs of real edges sorted by dst
    rdeg = deg - 1                            # real-edge in-degree per node
    ptr = np.zeros(N + 1, np.int64)
    ptr[1:] = np.cumsum(rdeg)

    degs_loc = deg.reshape(NCORES, NLOC)
    perm = np.argsort(-degs_loc, axis=1, kind="stable")   # dealt pos -> local node
    # slots per node = real-edge degree (self-loop handled densely)
    rdegs_loc = np.take_along_axis(rdeg.reshape(NCORES, NLOC), perm, 1)
    dsp = np.zeros((NCORES, NPAD), np.int64)
    dsp[:, :NLOC] = rdegs_loc
    D_common = dsp.reshape(NCORES, R, P).max(axis=(0, 2))
    D_common = np.maximum(D_common, 1)
    c0 = np.concatenate([[0], np.cumsum(D_common)]).astype(np.int64)
    CT = int(c0[-1])

    # global dealt-position map (node id -> dealt global position)
    dpg = np.empty(N, np.int64)
    node_ids = np.arange(NCORES)[:, None] * NLOC + perm
    dpg[node_ids] = np.arange(NCORES)[:, None] * NPAD + np.arange(NLOC)[None, :]

    def wrap_idx(a):
        # [P, CT] slot values -> dma_gather layout [128, 8*CT] int16
        streamT = np.ascontiguousarray(a.T).reshape(CT * 8, 16)
        return np.ascontiguousarray(np.tile(streamT.T, (8, 1)))

    cores = []
    for c in range(NCORES):
        nid = c * NLOC + perm[c]
        degn = rdeg[nid]
        tot = int(degn.sum())
        cum = np.zeros(NLOC + 1, np.int64)
        cum[1:] = np.cumsum(degn)
        kk = np.arange(tot, dtype=np.int64) - np.repeat(cum[:-1], degn)
        epos = np.repeat(ptr[nid], degn) + kk
        s_edge = ssrc[epos]
        noderep = np.repeat(np.arange(NLOC, dtype=np.int64), degn)
        chunk = c0[noderep // P] + kk
        part = noderep % P

        s2 = dpg[s_edge]
        idxv = np.zeros((P, CT), np.int16)
        idxv[part, chunk] = (s2 >> 2).astype(np.int16)
        # precomputed one-hot row-select mask duplicated in adjacent pairs,
        # [P, CT, 4, 2] -> [P, 8*CT] bf16 (packed innermost pair keeps the
        # DVE 2x 16-bit perf mode alive on the mask operand)
        import ml_dtypes
        mtv = np.zeros((P, CT, 4, 1), np.float32)
        mtv[part, chunk, (s2 & 3), 0] = 1.0
        mtv = np.repeat(mtv, 2, axis=3)
        msv = np.ascontiguousarray(
            mtv.reshape(P, 8 * CT).astype(ml_dtypes.bfloat16))

        import ml_dtypes
        x_pad = np.zeros((NPAD, FIN), np.float32)
        x_pad[:NLOC] = x[nid]                      # dealt order
        xt = np.ascontiguousarray(x_pad.T.astype(ml_dtypes.bfloat16))
        deg_t = np.ones((NPAD,), np.float32)
        deg_t[:NLOC] = deg[nid]
        deg_t = np.ascontiguousarray(deg_t.reshape(R, P).T)

        cores.append({
            "xt": xt,
            "deg": deg_t,
            "idx": wrap_idx(idxv),
            "ms": msv,
        })
    return D_common, perm, cores


def _make_calls(D_common):
    # Descending row order (thin rows first: rows are degree-sorted, so the
    # tail rows have the smallest D), with a small leading call so the first
    # gather's transfer primes the pipeline quickly at each layer start.
    calls = []
    r1 = R
    first_cap = 16
    while r1 > 0:
        cap = first_cap if not calls else CTILE
        acc = 0
        r0 = r1
        while r0 > 0 and acc + int(D_common[r0 - 1]) <= cap:
            r0 -= 1
            acc += int(D_common[r0])
        if r0 == r1:            # single row exceeds cap (fat rows)
            r0 = r1 - 1
            acc = int(D_common[r0])
        calls.append((r0, r1, acc))
        r1 = r0
    return calls


def kernel(x, edge_index, W1, b1, W2, b2):
    from concourse.bass_utils import run_bass_kernel_spmd

    x = np.asarray(x, np.float32)
    D_common, perm, cores = _preprocess(x, np.asarray(edge_index))
    calls = _make_calls(D_common)

    key = (tuple(int(v) for v in D_common), tuple(calls))
    if key not in _cache:
        _cache.clear()
        _cache[key] = _build_program(D_common, calls)
    nc = _cache[key]

    import ml_dtypes
    w1h = np.ascontiguousarray(np.asarray(W1, np.float32).astype(ml_dtypes.bfloat16))
    b1h = np.ascontiguousarray(np.tile(np.asarray(b1, np.float32)[None, :], (P, 1)))
    w2h = np.ascontiguousarray(np.asarray(W2, np.float32))
    b2h = np.ascontiguousarray(np.tile(np.asarray(b2, np.float32)[None, :], (P, 1)))
    in_maps = []
    for c in range(NCORES):
        m = dict(cores[c])
        m.update({"w1": w1h, "b1": b1h, "w2": w2h, "b2": b2h})
        in_maps.append(m)

    res = run_bass_kernel_spmd(nc, in_maps, core_ids=list(range(NCORES)))
    global last_results
    last_results = res

    out_full = np.empty((N, NCLS), np.float32)
    d = np.arange(NLOC)
    pp_ = d % P
    rr = d // P
    for c in range(NCORES):
        o = res.results[c]["out"]  # [P, R, NCLS]
        out_full[c * NLOC + perm[c]] = o[pp_, rr]
    return out_full
